# revision 1
# baseline (speedup 1.0000x reference)
"""Trainium2 Bass kernel for nn_AtomicNeuralNetwork (species-routed per-atom MLP).

Math (per frame n, atom a with species s = numbers[a]):
    h1 = silu(W1[s].T x + b1[s]);  h2 = silu(W2[s].T h1 + b2[s]);  out = W3[s].T h2 + b3[s]
Shapes: N=4096 frames, A=256 atoms, D_IN=39, H=50, S=8 species.

v2 strategy ("dense interleave"):
  - Data parallel over frames: 512 frames per NeuronCore x 8 cores.
  - Atoms are sorted by species and grouped into UNITS of 5 atoms. Each unit
    maps to 2 PSUM "regions" of [128 partitions x 512 frames]: region A holds
    hidden rows [a0(50) | a1(50) | a2 h0-24 (25)], region B holds
    [a3 | a4 | a2 h25-49]. 125/128 partitions carry useful data (vs 100/128
    for the naive 2-atoms-per-region layout), which cuts ScalarE (ACT)
    activation time - the bottleneck engine - by ~25%.
  - Matmuls use block-diagonal stationary weights built host-side per
    distinct species-5-tuple (sorting => few distinct patterns). L1: one
    [K=118, M=128] matmul per region (3 atom descs stacked + a ones-row).
    L2: one [126,128] matmul per region plus a small [30,30] cross-matmul at
    tile_position (96,96) accumulating the split atom's other-half
    contribution. L3: two [126,5] matmuls accumulate all 5 atom outputs.
  - ALL biases ride inside the matmuls via a constant-carrier row: dt row 117
    is 1.0, giving z1 += b1 directly; the stationary also produces
    ps row 125 = 8.0, and silu(8.0) rounds to exactly 8.0 in bf16, so
    the next layer's stationary row 125 = b/8 adds the bias (and 1.0 in
    col 125 re-emits the 8.0 carrier). No ACT bias, no DVE bias adds.
  - ACT does one Silu per region-pair [126 x 1024] straight out of PSUM.
  - L3 writes into the (already consumed) ps2 tile rows 0-4; one DVE
    tensor_copy per unit evacuates [5 x 512] to a 5-partition SBUF output
    collector (units side by side in columns); one DMA per 13 units.
  - PSUM: ps1 pool x2 + ps2 pool x2 of [128,1024] f32 = all 8 banks.
  - Everything on the matmul path is bf16 (PSUM accumulates fp32); desc is
    downcast to bf16 on the host (halves HBM traffic), and the split atom's
    desc block is shipped once (read by both regions' matmuls in place).

This rig is input-DMA-bound (~47-53 GB/s/core hard cap, 8x below spec), so
the queue plan matters most: A-inputs ride the SP HWDGE ring, B-inputs ride
SWDGE (gpsimd) with no compute-dependent DMAs in that FIFO, and outputs
ride the ACT ring (compute-paced anyway). Measured 259 us vs 311 us for
everything-on-one-queue, vs ~260 us for the previous quadrant-packed
baseline on the same device state (which also had ~10% more input bytes).
"""

import sys

for _p in ("/opt/trn_rl_repo",):
    if _p not in sys.path:
        sys.path.append(_p)

import numpy as np
import ml_dtypes

import concourse.bass as bass  # noqa: F401
import concourse.mybir as mybir
import concourse.tile as tile
from concourse import bacc
from concourse import bass_utils

N, A, D, H, S = 4096, 256, 39, 50, 8
NCORES = 8
NF = N // NCORES            # frames per core
MM_DT = mybir.dt.bfloat16
NP_MM = ml_dtypes.bfloat16

UA = 5                      # atoms per unit
KD = 3 * D + 1              # moving rows per region: 3 descs + ones = 118
G = 4                       # units per input-DMA group
OUT_U = 13                  # units per output collector tile / DMA
DEDUP = True                # don't ship the split atom's desc twice
QMODE = "split3b"           # A-in->SP, B-in->SWDGE (1/4 to SP), outs->ACT
KB = 2 * D + 1              # dedup: B-region moving rows: 2 descs + ones = 79
CAR = 8.0                   # bias carrier: bf16(silu(8.0)) == 8.0 exactly
WSEG = 618                  # stationary image cols per species pattern

LAST = {}


def _units(numbers):
    """Sort atoms by species, pad to a multiple of UA*G with dups of the last
    atom. Returns (slots [nslot], valid [nslot], unit_pat [nunit], patterns)."""
    order = np.argsort(numbers, kind="stable").astype(np.int64)
    nunit = -(-A // UA)
    nunit = -(-nunit // G) * G                    # multiple of G
    nslot = UA * nunit
    slots = np.concatenate([order, np.full(nslot - A, order[-1], np.int64)])
    valid = np.zeros(nslot, bool)
    valid[:A] = True
    sp5 = np.asarray(numbers)[slots].reshape(nunit, UA)
    patterns = {}
    unit_pat = []
    for u in range(nunit):
        key = tuple(int(x) for x in sp5[u])
        if key not in patterns:
            patterns[key] = len(patterns)
        unit_pat.append(patterns[key])
    return slots, valid, unit_pat, list(patterns.keys())


def _weight_images(pats, W1, b1, W2, b2, W3, b3):
    """[128, WSEG*npat] f32 stationary image; see module docstring."""
    npat = len(pats)
    img = np.zeros((128, WSEG * npat), np.float32)
    for p, (t0, t1, t2, t3, t4) in enumerate(pats):
        c = p * WSEG
        # S1A [0:128): K rows = [desc a2 | a0 | a1 | ones], M cols = z1A
        img[0:39, c + 100:c + 125] = W1[t2][:, 0:25]
        img[39:78, c + 0:c + 50] = W1[t0]
        img[78:117, c + 50:c + 100] = W1[t1]
        img[117, c + 0:c + 50] = b1[t0]
        img[117, c + 50:c + 100] = b1[t1]
        img[117, c + 100:c + 125] = b1[t2][0:25]
        img[117, c + 125] = CAR
        # S1B [128:256): K rows = [desc a3 | a4 | ones(row 78)], M = z1B
        c1 = c + 128
        img[0:39, c1 + 0:c1 + 50] = W1[t3]
        img[39:78, c1 + 50:c1 + 100] = W1[t4]
        img[78, c1 + 0:c1 + 50] = b1[t3]
        img[78, c1 + 50:c1 + 100] = b1[t4]
        img[78, c1 + 100:c1 + 125] = b1[t2][25:50]
        img[78, c1 + 125] = CAR
        # S1BX [588:618): a2's z1B part from the A-region desc rows 0:39
        c8 = c + 588
        img[0:39, c8 + 4:c8 + 29] = W1[t2][:, 25:50]
        # S2A [256:384): K rows = h1A layout, M = z2A layout
        c2 = c + 256
        img[0:50, c2 + 0:c2 + 50] = W2[t0]
        img[50:100, c2 + 50:c2 + 100] = W2[t1]
        img[100:125, c2 + 100:c2 + 125] = W2[t2][0:25, 0:25]
        img[125, c2 + 0:c2 + 50] = b2[t0] / CAR
        img[125, c2 + 50:c2 + 100] = b2[t1] / CAR
        img[125, c2 + 100:c2 + 125] = b2[t2][0:25] / CAR
        img[125, c2 + 125] = 1.0                  # re-emit carrier
        # S2B [384:512)
        c3 = c + 384
        img[0:50, c3 + 0:c3 + 50] = W2[t3]
        img[50:100, c3 + 50:c3 + 100] = W2[t4]
        img[100:125, c3 + 100:c3 + 125] = W2[t2][25:50, 25:50]
        img[125, c3 + 0:c3 + 50] = b2[t3] / CAR
        img[125, c3 + 50:c3 + 100] = b2[t4] / CAR
        img[125, c3 + 100:c3 + 125] = b2[t2][25:50] / CAR
        img[125, c3 + 125] = 1.0
        # S2AX [512:542): rows 96+j (h1B), cols 4+o -> z2A rows 100+o
        c4 = c + 512
        img[100:125, c4 + 4:c4 + 29] = W2[t2][25:50, 0:25]
        # S2BX [542:572): rows 96+j (h1A), cols 4+o -> z2B rows 100+o
        c5 = c + 542
        img[100:125, c5 + 4:c5 + 29] = W2[t2][0:25, 25:50]
        # S3A [572:580): cols = atoms 0..4 from h2A
        c6 = c + 572
        img[0:50, c6 + 0] = W3[t0][:, 0]
        img[50:100, c6 + 1] = W3[t1][:, 0]
        img[100:125, c6 + 2] = W3[t2][0:25, 0]
        img[125, c6 + 0:c6 + 5] = b3[[t0, t1, t2, t3, t4], 0] / CAR
        # S3B [580:588): cols = atoms 0..4 from h2B
        c7 = c + 580
        img[100:125, c7 + 2] = W3[t2][25:50, 0]
        img[0:50, c7 + 3] = W3[t3][:, 0]
        img[50:100, c7 + 4] = W3[t4][:, 0]
    return img


def _prepare(desc, numbers, W1, b1, W2, b2, W3, b3):
    desc = np.asarray(desc, np.float32)
    numbers = np.asarray(numbers).astype(np.int64)
    W1 = np.asarray(W1, np.float32); b1 = np.asarray(b1, np.float32)
    W2 = np.asarray(W2, np.float32); b2 = np.asarray(b2, np.float32)
    W3 = np.asarray(W3, np.float32); b3 = np.asarray(b3, np.float32)

    slots, valid, unit_pat, pats = _units(numbers)
    nunit = len(unit_pat)
    ngrp = nunit // G
    wimg = _weight_images(pats, W1, b1, W2, b2, W3, b3).astype(NP_MM)

    sela = np.empty((nunit, 3), np.int64)
    selb = np.empty((nunit, 2), np.int64)
    for u in range(nunit):
        sela[u] = (5 * u + 2, 5 * u, 5 * u + 1)     # [a2 | a0 | a1]
        selb[u] = (5 * u + 3, 5 * u + 4)

    in_maps = []
    for c in range(NCORES):
        at = desc[c * NF:(c + 1) * NF][:, slots, :]          # [NF, nslot, D]
        at = np.ascontiguousarray(at.transpose(1, 2, 0))     # [nslot, D, NF]
        da = np.empty((nunit, KD, NF), np.float32)
        da[:, 0:3 * D] = at[sela.reshape(-1)].reshape(nunit, 3 * D, NF)
        da[:, 3 * D] = 1.0
        db = np.empty((nunit, KB, NF), np.float32)
        db[:, 0:2 * D] = at[selb.reshape(-1)].reshape(nunit, 2 * D, NF)
        db[:, 2 * D] = 1.0
        da = da.astype(NP_MM).reshape(ngrp, G, KD, NF)
        da = np.ascontiguousarray(da.transpose(0, 2, 1, 3)).reshape(ngrp, KD, G * NF)
        db = db.astype(NP_MM).reshape(ngrp, G, KB, NF)
        db = np.ascontiguousarray(db.transpose(0, 2, 1, 3)).reshape(ngrp, KB, G * NF)
        in_maps.append({"desc_a": da, "desc_b": db, "wt_in": wimg})

    meta = dict(unit_pat=unit_pat, npat=len(pats), nunit=nunit,
                slots=slots, valid=valid)
    return in_maps, meta


def _build(meta, repeat=0):
    import contextlib

    unit_pat = meta["unit_pat"]
    npat = meta["npat"]
    nunit = meta["nunit"]
    ngrp = nunit // G

    nc = bacc.Bacc("TRN2", target_bir_lowering=False, debug=False)
    desc_a = nc.dram_tensor("desc_a", [ngrp, KD, G * NF], MM_DT,
                            kind="ExternalInput")
    desc_b = nc.dram_tensor("desc_b", [ngrp, KB, G * NF], MM_DT,
                            kind="ExternalInput")
    wt_in = nc.dram_tensor("wt_in", [128, WSEG * npat], MM_DT,
                           kind="ExternalInput")
    notile = -(-nunit // OUT_U)
    out = nc.dram_tensor("out", [notile, UA, OUT_U * NF], mybir.dt.float32,
                         kind="ExternalOutput")

    Silu = mybir.ActivationFunctionType.Silu
    F32 = mybir.dt.float32

    with tile.TileContext(nc) as tc:
        with (
            tc.tile_pool(name="w", bufs=1) as wpool,
            tc.tile_pool(name="dt", bufs=3) as dpool,
            tc.tile_pool(name="h1", bufs=3) as h1pool,
            tc.tile_pool(name="h2", bufs=3) as h2pool,
            tc.tile_pool(name="o", bufs=2) as opool,
            tc.tile_pool(name="ps1", bufs=2, space="PSUM") as ps1pool,
            tc.tile_pool(name="ps2", bufs=2, space="PSUM") as ps2pool,
        ):
            wt = wpool.tile([128, WSEG * npat], MM_DT)
            nc.sync.dma_start(wt[:], wt_in[:])

            # DMA queue strategy (QMODE): measured in-kernel, the SP HWDGE
            # ring alone sustains ~50 GB/s; leaning on the ACT ring or SWDGE
            # while compute runs measured WORSE despite idle-bench gains.
            if QMODE == "greedy":
                _rate = {"sync": 52.8, "gpsimd": 22.6}
                _load = {"sync": 0.0, "gpsimd": 0.0}
                _eng = {"sync": nc.sync, "gpsimd": nc.gpsimd}

                def pick_queue(nbytes):
                    q = min(_rate, key=lambda k: (_load[k] + nbytes) / _rate[k])
                    _load[q] += nbytes
                    return _eng[q]
            elif QMODE == "sync+scalar":
                _n = [0]

                def pick_queue(nbytes):
                    _n[0] += 1
                    return nc.scalar if _n[0] % 2 else nc.sync
            else:

                def pick_queue(nbytes):
                    return nc.sync

            if QMODE in ("split3", "split3b"):
                # dedicate queues by dependency class: A-input -> SP ring,
                # B-input -> SWDGE (input-only, never gated behind compute),
                # outs -> ACT ring (compute-paced anyway). split3b also
                # rebalances ~1MB of B back to SP so SWDGE (~22.6 GB/s
                # marginal) and SP (~53 GB/s) finish together.
                _bn = [0]

                def pick_queue(nbytes, kind="a"):
                    if kind == "b":
                        _bn[0] += 1
                        if QMODE == "split3b" and _bn[0] % 4 == 0:
                            return nc.sync
                        return nc.gpsimd
                    return {"a": nc.sync, "o": nc.scalar}[kind]
            else:
                _pq = pick_queue

                def pick_queue(nbytes, kind="a"):
                    return _pq(nbytes)

            loop_cm = tc.For_i(0, repeat, 1) if repeat else contextlib.nullcontext()
            with loop_cm:
                dt_box = [None]

                def emit_l1(u):
                    """L1 matmuls for unit u (emitted one unit ahead so the
                    PE can fill ps1(u+1) while ACT runs ACT2(u) - keeps ACT
                    from idling on the L3(u)->L1(u+1) chain)."""
                    g, j = divmod(u, G)
                    if j == 0:
                        dt_new = dpool.tile([128, 2 * G * NF], MM_DT, tag="dt")
                        pick_queue(KD * G * NF * 2, "a").dma_start(
                            dt_new[0:KD, 0:G * NF], desc_a[g, :, :])
                        pick_queue(KB * G * NF * 2, "b").dma_start(
                            dt_new[0:KB, G * NF:2 * G * NF], desc_b[g, :, :])
                        dt_box[0] = dt_new
                    dt_t = dt_box[0]
                    coa = j * NF
                    cob = (G + j) * NF
                    b = unit_pat[u] * WSEG
                    ps1 = ps1pool.tile([128, 2 * NF], F32, tag="ps1")
                    nc.tensor.matmul(ps1[0:128, 0:NF], wt[0:KD, b:b + 128],
                                     dt_t[0:KD, coa:coa + NF],
                                     start=True, stop=True)
                    nc.tensor.matmul(ps1[0:128, NF:2 * NF],
                                     wt[0:KB, b + 128:b + 256],
                                     dt_t[0:KB, cob:cob + NF],
                                     start=True, stop=False)
                    nc.tensor.matmul(ps1[96:126, NF:2 * NF],
                                     wt[0:D, b + 588:b + 618],
                                     dt_t[0:D, coa:coa + NF],
                                     start=False, stop=True,
                                     tile_position=(0, 96))
                    return ps1

                osb = None
                ps1 = emit_l1(0)
                for u in range(nunit):
                    if u % OUT_U == 0:
                        osb = opool.tile([UA, OUT_U * NF], F32, tag="o")
                    b = unit_pat[u] * WSEG

                    h1 = h1pool.tile([128, 2 * NF], MM_DT, tag="h1")
                    nc.scalar.activation(h1[0:126, :], ps1[0:126, :], Silu)

                    # ---- L2: two mains + two split-atom cross terms ----
                    ps2 = ps2pool.tile([128, 2 * NF], F32, tag="ps2")
                    nc.tensor.matmul(ps2[0:128, 0:NF], wt[0:126, b + 256:b + 384],
                                     h1[0:126, 0:NF], start=True, stop=False)
                    nc.tensor.matmul(ps2[0:128, NF:2 * NF],
                                     wt[0:126, b + 384:b + 512],
                                     h1[0:126, NF:2 * NF], start=True, stop=False)
                    nc.tensor.matmul(ps2[96:126, 0:NF],
                                     wt[96:126, b + 512:b + 542],
                                     h1[96:126, NF:2 * NF],
                                     start=False, stop=True,
                                     tile_position=(96, 96))
                    nc.tensor.matmul(ps2[96:126, NF:2 * NF],
                                     wt[96:126, b + 542:b + 572],
                                     h1[96:126, 0:NF],
                                     start=False, stop=True,
                                     tile_position=(96, 96))

                    if u + 1 < nunit:
                        ps1 = emit_l1(u + 1)

                    h2 = h2pool.tile([128, 2 * NF], MM_DT, tag="h2")
                    nc.scalar.activation(h2[0:126, :], ps2[0:126, :], Silu)

                    # ---- L3 into the consumed ps2 rows 0-4 ----
                    nc.tensor.matmul(ps2[0:5, 0:NF], wt[0:126, b + 572:b + 577],
                                     h2[0:126, 0:NF], start=True, stop=False)
                    nc.tensor.matmul(ps2[0:5, 0:NF], wt[0:126, b + 580:b + 585],
                                     h2[0:126, NF:2 * NF], start=False, stop=True)
                    co_o = (u % OUT_U) * NF
                    nc.vector.tensor_copy(osb[0:UA, co_o:co_o + NF],
                                          ps2[0:UA, 0:NF])
                    if u % OUT_U == OUT_U - 1 or u == nunit - 1:
                        t = u // OUT_U
                        pick_queue(UA * (co_o + NF) * 4, "o").dma_start(
                            out[t, :, 0:co_o + NF], osb[0:UA, 0:co_o + NF])

    nc.compile()
    return nc


def kernel(desc, numbers, W1, b1, W2, b2, W3, b3):
    in_maps, meta = _prepare(desc, numbers, W1, b1, W2, b2, W3, b3)
    nc = _build(meta)

    last_err = None
    for _attempt in range(3):
        try:
            res = bass_utils.run_bass_kernel_spmd(
                nc, in_maps, core_ids=list(range(NCORES)))
            break
        except Exception as e:  # transient axon terminal failures
            last_err = e
            import time
            time.sleep(20)
    else:
        raise last_err

    LAST.update(nc=nc, in_maps=in_maps, res=res, meta=meta)

    slots, valid = meta["slots"], meta["valid"]
    out = np.empty((N, A), np.float32)
    for c in range(NCORES):
        oc = res.results[c]["out"]                   # [notile, UA, OUT_U*NF]
        oc = oc.reshape(-1, UA, OUT_U, NF)           # [t, a, u_in_tile, NF]
        oc = oc.transpose(0, 2, 1, 3).reshape(-1, NF)  # slot-major [nslot', NF]
        nslot = len(slots)
        out[c * NF:(c + 1) * NF, slots[valid]] = oc[:nslot][valid].T
    return out



# revision 2
# speedup vs baseline: 1.6496x; 1.6496x over previous
"""Trainium2 Bass kernel for nn_AtomicNeuralNetwork (species-routed per-atom MLP).

Math (per frame n, atom a with species s = numbers[a]):
    h1 = silu(W1[s].T x + b1[s]);  h2 = silu(W2[s].T h1 + b2[s]);  out = W3[s].T h2 + b3[s]
Shapes: N=4096 frames, A=256 atoms, D_IN=39, H=50, S=8 species.

v2 strategy ("dense interleave"):
  - Data parallel over frames: 512 frames per NeuronCore x 8 cores.
  - Atoms are sorted by species and grouped into UNITS of 5 atoms. Each unit
    maps to 2 PSUM "regions" of [128 partitions x 512 frames]: region A holds
    hidden rows [a0(50) | a1(50) | a2 h0-24 (25)], region B holds
    [a3 | a4 | a2 h25-49]. 125/128 partitions carry useful data (vs 100/128
    for the naive 2-atoms-per-region layout), which cuts ScalarE (ACT)
    activation time - the bottleneck engine - by ~25%.
  - Matmuls use block-diagonal stationary weights built host-side per
    distinct species-5-tuple (sorting => few distinct patterns). L1: one
    [K=118, M=128] matmul per region (3 atom descs stacked + a ones-row).
    L2: one [126,128] matmul per region plus a small [30,30] cross-matmul at
    tile_position (96,96) accumulating the split atom's other-half
    contribution. L3: two [126,5] matmuls accumulate all 5 atom outputs.
  - ALL biases ride inside the matmuls via a constant-carrier row: dt row 117
    is 1.0, giving z1 += b1 directly; the stationary also produces
    ps row 125 = 8.0, and silu(8.0) rounds to exactly 8.0 in bf16, so
    the next layer's stationary row 125 = b/8 adds the bias (and 1.0 in
    col 125 re-emits the 8.0 carrier). No ACT bias, no DVE bias adds.
  - ACT does one Silu per region-pair [126 x 1024] straight out of PSUM.
  - L3 writes into the (already consumed) ps2 tile rows 0-4; one DVE
    tensor_copy per unit evacuates [5 x 512] to a 5-partition SBUF output
    collector (units side by side in columns); one DMA per 13 units.
  - PSUM: ps1 pool x2 + ps2 pool x2 of [128,1024] f32 = all 8 banks.
  - Everything on the matmul path is bf16 (PSUM accumulates fp32); desc is
    downcast to bf16 on the host (halves HBM traffic), and the split atom's
    desc block is shipped once (read by both regions' matmuls in place).

This rig is input-DMA-bound (~47-53 GB/s/core hard cap, 8x below spec), so
the queue plan matters most: A-inputs ride the SP HWDGE ring, B-inputs ride
SWDGE (gpsimd) with no compute-dependent DMAs in that FIFO, and outputs
ride the ACT ring (compute-paced anyway). Measured 259 us vs 311 us for
everything-on-one-queue, vs ~260 us for the previous quadrant-packed
baseline on the same device state (which also had ~10% more input bytes).
"""

import sys

for _p in ("/opt/trn_rl_repo",):
    if _p not in sys.path:
        sys.path.append(_p)

import numpy as np
import ml_dtypes

import concourse.bass as bass  # noqa: F401
import concourse.mybir as mybir
import concourse.tile as tile
from concourse import bacc
from concourse import bass_utils

N, A, D, H, S = 4096, 256, 39, 50, 8
NCORES = 8
NF = N // NCORES            # frames per core
MM_DT = mybir.dt.bfloat16
NP_MM = ml_dtypes.bfloat16

UA = 5                      # atoms per unit
KD = 3 * D + 1              # moving rows per region: 3 descs + ones = 118
G = 4                       # units per input-DMA group
OUT_U = 13                  # units per output collector tile / DMA
DEDUP = True                # don't ship the split atom's desc twice
QMODE = "split3b"
SKIP_INPUT_DMA = False           # A-in->SP, B-in->SWDGE (1/4 to SP), outs->ACT
KB = 2 * D + 1              # dedup: B-region moving rows: 2 descs + ones = 79
CAR = 8.0                   # bias carrier: bf16(silu(8.0)) == 8.0 exactly
WSEG = 618                  # stationary image cols per species pattern

LAST = {}


def _units(numbers):
    """Sort atoms by species, pad to a multiple of UA*G with dups of the last
    atom. Returns (slots [nslot], valid [nslot], unit_pat [nunit], patterns)."""
    order = np.argsort(numbers, kind="stable").astype(np.int64)
    nunit = -(-A // UA)
    nunit = -(-nunit // G) * G                    # multiple of G
    nslot = UA * nunit
    slots = np.concatenate([order, np.full(nslot - A, order[-1], np.int64)])
    valid = np.zeros(nslot, bool)
    valid[:A] = True
    sp5 = np.asarray(numbers)[slots].reshape(nunit, UA)
    patterns = {}
    unit_pat = []
    for u in range(nunit):
        key = tuple(int(x) for x in sp5[u])
        if key not in patterns:
            patterns[key] = len(patterns)
        unit_pat.append(patterns[key])
    return slots, valid, unit_pat, list(patterns.keys())


def _weight_images(pats, W1, b1, W2, b2, W3, b3):
    """[128, WSEG*npat] f32 stationary image; see module docstring."""
    npat = len(pats)
    img = np.zeros((128, WSEG * npat), np.float32)
    for p, (t0, t1, t2, t3, t4) in enumerate(pats):
        c = p * WSEG
        # S1A [0:128): K rows = [desc a2 | a0 | a1 | ones], M cols = z1A
        img[0:39, c + 100:c + 125] = W1[t2][:, 0:25]
        img[39:78, c + 0:c + 50] = W1[t0]
        img[78:117, c + 50:c + 100] = W1[t1]
        img[117, c + 0:c + 50] = b1[t0]
        img[117, c + 50:c + 100] = b1[t1]
        img[117, c + 100:c + 125] = b1[t2][0:25]
        img[117, c + 125] = CAR
        # S1B [128:256): K rows = [desc a3 | a4 | ones(row 78)], M = z1B
        c1 = c + 128
        img[0:39, c1 + 0:c1 + 50] = W1[t3]
        img[39:78, c1 + 50:c1 + 100] = W1[t4]
        img[78, c1 + 0:c1 + 50] = b1[t3]
        img[78, c1 + 50:c1 + 100] = b1[t4]
        img[78, c1 + 100:c1 + 125] = b1[t2][25:50]
        img[78, c1 + 125] = CAR
        # S1BX [588:618): a2's z1B part from the A-region desc rows 0:39
        c8 = c + 588
        img[0:39, c8 + 4:c8 + 29] = W1[t2][:, 25:50]
        # S2A [256:384): K rows = h1A layout, M = z2A layout
        c2 = c + 256
        img[0:50, c2 + 0:c2 + 50] = W2[t0]
        img[50:100, c2 + 50:c2 + 100] = W2[t1]
        img[100:125, c2 + 100:c2 + 125] = W2[t2][0:25, 0:25]
        img[125, c2 + 0:c2 + 50] = b2[t0] / CAR
        img[125, c2 + 50:c2 + 100] = b2[t1] / CAR
        img[125, c2 + 100:c2 + 125] = b2[t2][0:25] / CAR
        img[125, c2 + 125] = 1.0                  # re-emit carrier
        # S2B [384:512)
        c3 = c + 384
        img[0:50, c3 + 0:c3 + 50] = W2[t3]
        img[50:100, c3 + 50:c3 + 100] = W2[t4]
        img[100:125, c3 + 100:c3 + 125] = W2[t2][25:50, 25:50]
        img[125, c3 + 0:c3 + 50] = b2[t3] / CAR
        img[125, c3 + 50:c3 + 100] = b2[t4] / CAR
        img[125, c3 + 100:c3 + 125] = b2[t2][25:50] / CAR
        img[125, c3 + 125] = 1.0
        # S2AX [512:542): rows 96+j (h1B), cols 4+o -> z2A rows 100+o
        c4 = c + 512
        img[100:125, c4 + 4:c4 + 29] = W2[t2][25:50, 0:25]
        # S2BX [542:572): rows 96+j (h1A), cols 4+o -> z2B rows 100+o
        c5 = c + 542
        img[100:125, c5 + 4:c5 + 29] = W2[t2][0:25, 25:50]
        # S3A [572:580): cols = atoms 0..4 from h2A
        c6 = c + 572
        img[0:50, c6 + 0] = W3[t0][:, 0]
        img[50:100, c6 + 1] = W3[t1][:, 0]
        img[100:125, c6 + 2] = W3[t2][0:25, 0]
        img[125, c6 + 0:c6 + 5] = b3[[t0, t1, t2, t3, t4], 0] / CAR
        # S3B [580:588): cols = atoms 0..4 from h2B
        c7 = c + 580
        img[100:125, c7 + 2] = W3[t2][25:50, 0]
        img[0:50, c7 + 3] = W3[t3][:, 0]
        img[50:100, c7 + 4] = W3[t4][:, 0]
    return img


def _prepare(desc, numbers, W1, b1, W2, b2, W3, b3):
    desc = np.asarray(desc, np.float32)
    numbers = np.asarray(numbers).astype(np.int64)
    W1 = np.asarray(W1, np.float32); b1 = np.asarray(b1, np.float32)
    W2 = np.asarray(W2, np.float32); b2 = np.asarray(b2, np.float32)
    W3 = np.asarray(W3, np.float32); b3 = np.asarray(b3, np.float32)

    slots, valid, unit_pat, pats = _units(numbers)
    nunit = len(unit_pat)
    ngrp = nunit // G
    wimg = _weight_images(pats, W1, b1, W2, b2, W3, b3).astype(NP_MM)

    sela = np.empty((nunit, 3), np.int64)
    selb = np.empty((nunit, 2), np.int64)
    for u in range(nunit):
        sela[u] = (5 * u + 2, 5 * u, 5 * u + 1)     # [a2 | a0 | a1]
        selb[u] = (5 * u + 3, 5 * u + 4)

    in_maps = []
    for c in range(NCORES):
        at = desc[c * NF:(c + 1) * NF][:, slots, :]          # [NF, nslot, D]
        at = np.ascontiguousarray(at.transpose(1, 2, 0))     # [nslot, D, NF]
        da = np.empty((nunit, KD, NF), np.float32)
        da[:, 0:3 * D] = at[sela.reshape(-1)].reshape(nunit, 3 * D, NF)
        da[:, 3 * D] = 1.0
        db = np.empty((nunit, KB, NF), np.float32)
        db[:, 0:2 * D] = at[selb.reshape(-1)].reshape(nunit, 2 * D, NF)
        db[:, 2 * D] = 1.0
        da = da.astype(NP_MM).reshape(ngrp, G, KD, NF)
        da = np.ascontiguousarray(da.transpose(0, 2, 1, 3)).reshape(ngrp, KD, G * NF)
        db = db.astype(NP_MM).reshape(ngrp, G, KB, NF)
        db = np.ascontiguousarray(db.transpose(0, 2, 1, 3)).reshape(ngrp, KB, G * NF)
        in_maps.append({"desc_a": da, "desc_b": db, "wt_in": wimg})

    meta = dict(unit_pat=unit_pat, npat=len(pats), nunit=nunit,
                slots=slots, valid=valid)
    return in_maps, meta


def _build(meta, repeat=0):
    import contextlib

    unit_pat = meta["unit_pat"]
    npat = meta["npat"]
    nunit = meta["nunit"]
    ngrp = nunit // G

    nc = bacc.Bacc("TRN2", target_bir_lowering=False, debug=False)
    desc_a = nc.dram_tensor("desc_a", [ngrp, KD, G * NF], MM_DT,
                            kind="ExternalInput")
    desc_b = nc.dram_tensor("desc_b", [ngrp, KB, G * NF], MM_DT,
                            kind="ExternalInput")
    wt_in = nc.dram_tensor("wt_in", [128, WSEG * npat], MM_DT,
                           kind="ExternalInput")
    notile = -(-nunit // OUT_U)
    out = nc.dram_tensor("out", [notile, UA, OUT_U * NF], mybir.dt.float32,
                         kind="ExternalOutput")

    Silu = mybir.ActivationFunctionType.Silu
    F32 = mybir.dt.float32

    with tile.TileContext(nc) as tc:
        with (
            tc.tile_pool(name="w", bufs=1) as wpool,
            tc.tile_pool(name="dt", bufs=3) as dpool,
            tc.tile_pool(name="h1", bufs=3) as h1pool,
            tc.tile_pool(name="h2", bufs=3) as h2pool,
            tc.tile_pool(name="o", bufs=2) as opool,
            tc.tile_pool(name="ps1", bufs=2, space="PSUM") as ps1pool,
            tc.tile_pool(name="ps2", bufs=2, space="PSUM") as ps2pool,
        ):
            wt = wpool.tile([128, WSEG * npat], MM_DT)
            nc.sync.dma_start(wt[:], wt_in[:])

            # DMA queue strategy (QMODE): measured in-kernel, the SP HWDGE
            # ring alone sustains ~50 GB/s; leaning on the ACT ring or SWDGE
            # while compute runs measured WORSE despite idle-bench gains.
            if QMODE == "greedy":
                _rate = {"sync": 52.8, "gpsimd": 22.6}
                _load = {"sync": 0.0, "gpsimd": 0.0}
                _eng = {"sync": nc.sync, "gpsimd": nc.gpsimd}

                def pick_queue(nbytes):
                    q = min(_rate, key=lambda k: (_load[k] + nbytes) / _rate[k])
                    _load[q] += nbytes
                    return _eng[q]
            elif QMODE == "sync+scalar":
                _n = [0]

                def pick_queue(nbytes):
                    _n[0] += 1
                    return nc.scalar if _n[0] % 2 else nc.sync
            else:

                def pick_queue(nbytes):
                    return nc.sync

            if QMODE in ("split3", "split3b"):
                # dedicate queues by dependency class: A-input -> SP ring,
                # B-input -> SWDGE (input-only, never gated behind compute),
                # outs -> ACT ring (compute-paced anyway). split3b also
                # rebalances ~1MB of B back to SP so SWDGE (~22.6 GB/s
                # marginal) and SP (~53 GB/s) finish together.
                _bn = [0]

                def pick_queue(nbytes, kind="a"):
                    if kind == "b":
                        _bn[0] += 1
                        if QMODE == "split3b" and _bn[0] % 4 == 0:
                            return nc.sync
                        return nc.gpsimd
                    return {"a": nc.sync, "o": nc.scalar}[kind]
            else:
                _pq = pick_queue

                def pick_queue(nbytes, kind="a"):
                    return _pq(nbytes)

            loop_cm = tc.For_i(0, repeat, 1) if repeat else contextlib.nullcontext()
            with loop_cm:
                dt_box = [None]

                def emit_l1(u):
                    """L1 matmuls for unit u (emitted one unit ahead so the
                    PE can fill ps1(u+1) while ACT runs ACT2(u) - keeps ACT
                    from idling on the L3(u)->L1(u+1) chain)."""
                    g, j = divmod(u, G)
                    if j == 0:
                        dt_new = dpool.tile([128, 2 * G * NF], MM_DT, tag="dt")
                        if SKIP_INPUT_DMA:
                            pick_queue(0, "a").dma_start(
                                dt_new[0:KD, 0:16], desc_a[g, :, 0:16])
                            pick_queue(0, "b").dma_start(
                                dt_new[0:KB, G * NF:G * NF + 16],
                                desc_b[g, :, 0:16])
                        else:
                            pick_queue(KD * G * NF * 2, "a").dma_start(
                                dt_new[0:KD, 0:G * NF], desc_a[g, :, :])
                            pick_queue(KB * G * NF * 2, "b").dma_start(
                                dt_new[0:KB, G * NF:2 * G * NF], desc_b[g, :, :])
                        dt_box[0] = dt_new
                    dt_t = dt_box[0]
                    coa = j * NF
                    cob = (G + j) * NF
                    b = unit_pat[u] * WSEG
                    ps1 = ps1pool.tile([128, 2 * NF], F32, tag="ps1")
                    nc.tensor.matmul(ps1[0:128, 0:NF], wt[0:KD, b:b + 128],
                                     dt_t[0:KD, coa:coa + NF],
                                     start=True, stop=True)
                    nc.tensor.matmul(ps1[0:128, NF:2 * NF],
                                     wt[0:KB, b + 128:b + 256],
                                     dt_t[0:KB, cob:cob + NF],
                                     start=True, stop=False)
                    nc.tensor.matmul(ps1[96:126, NF:2 * NF],
                                     wt[0:D, b + 588:b + 618],
                                     dt_t[0:D, coa:coa + NF],
                                     start=False, stop=True,
                                     tile_position=(0, 96))
                    return ps1

                osb = None
                ps1 = emit_l1(0)
                for u in range(nunit):
                    if u % OUT_U == 0:
                        osb = opool.tile([UA, OUT_U * NF], F32, tag="o")
                    b = unit_pat[u] * WSEG

                    h1 = h1pool.tile([128, 2 * NF], MM_DT, tag="h1")
                    nc.scalar.activation(h1[0:126, :], ps1[0:126, :], Silu)

                    # ---- L2: two mains + two split-atom cross terms ----
                    ps2 = ps2pool.tile([128, 2 * NF], F32, tag="ps2")
                    nc.tensor.matmul(ps2[0:128, 0:NF], wt[0:126, b + 256:b + 384],
                                     h1[0:126, 0:NF], start=True, stop=False)
                    nc.tensor.matmul(ps2[0:128, NF:2 * NF],
                                     wt[0:126, b + 384:b + 512],
                                     h1[0:126, NF:2 * NF], start=True, stop=False)
                    nc.tensor.matmul(ps2[96:126, 0:NF],
                                     wt[96:126, b + 512:b + 542],
                                     h1[96:126, NF:2 * NF],
                                     start=False, stop=True,
                                     tile_position=(96, 96))
                    nc.tensor.matmul(ps2[96:126, NF:2 * NF],
                                     wt[96:126, b + 542:b + 572],
                                     h1[96:126, 0:NF],
                                     start=False, stop=True,
                                     tile_position=(96, 96))

                    if u + 1 < nunit:
                        ps1 = emit_l1(u + 1)

                    h2 = h2pool.tile([128, 2 * NF], MM_DT, tag="h2")
                    nc.scalar.activation(h2[0:126, :], ps2[0:126, :], Silu)

                    # ---- L3 into the consumed ps2 rows 0-4 ----
                    nc.tensor.matmul(ps2[0:5, 0:NF], wt[0:126, b + 572:b + 577],
                                     h2[0:126, 0:NF], start=True, stop=False)
                    nc.tensor.matmul(ps2[0:5, 0:NF], wt[0:126, b + 580:b + 585],
                                     h2[0:126, NF:2 * NF], start=False, stop=True)
                    co_o = (u % OUT_U) * NF
                    nc.vector.tensor_copy(osb[0:UA, co_o:co_o + NF],
                                          ps2[0:UA, 0:NF])
                    if u % OUT_U == OUT_U - 1 or u == nunit - 1:
                        t = u // OUT_U
                        pick_queue(UA * (co_o + NF) * 4, "o").dma_start(
                            out[t, :, 0:co_o + NF], osb[0:UA, 0:co_o + NF])

    nc.compile()
    return nc


def kernel(desc, numbers, W1, b1, W2, b2, W3, b3):
    in_maps, meta = _prepare(desc, numbers, W1, b1, W2, b2, W3, b3)
    nc = _build(meta)

    last_err = None
    for _attempt in range(3):
        try:
            res = bass_utils.run_bass_kernel_spmd(
                nc, in_maps, core_ids=list(range(NCORES)))
            break
        except Exception as e:  # transient axon terminal failures
            last_err = e
            import time
            time.sleep(20)
    else:
        raise last_err

    LAST.update(nc=nc, in_maps=in_maps, res=res, meta=meta)

    slots, valid = meta["slots"], meta["valid"]
    out = np.empty((N, A), np.float32)
    for c in range(NCORES):
        oc = res.results[c]["out"]                   # [notile, UA, OUT_U*NF]
        oc = oc.reshape(-1, UA, OUT_U, NF)           # [t, a, u_in_tile, NF]
        oc = oc.transpose(0, 2, 1, 3).reshape(-1, NF)  # slot-major [nslot', NF]
        nslot = len(slots)
        out[c * NF:(c + 1) * NF, slots[valid]] = oc[:nslot][valid].T
    return out

